# revision 21
# baseline (speedup 1.0000x reference)
"""GCN-with-edge-MLP kernel for trn2, 8-core SPMD (self-contained).

Equal node shards (12544 nodes/core, row = node id everywhere). Per core:
  conv1: per 128-node dst tile, slots (edges+self, grouped by src quarter
         window so indices fit int16) gathered from x fp32 rows (256B) via
         4-queue dma_gather, scaled by host norm (dinv_s*dinv_d) on the Pool
         engine, aggregated with one-hot (is_equal) matmuls on PE ->
         h1 = relu(agg@W1 + b1), q = h1 @ (W2@fcW1[:64]).
  AllGather(q); expand q into 256B-strided quarter tables (strided DMA).
  conv2: same slot structure and the SAME int16 indices, gather q rows,
         2-wide aggregation -> p. Fused pd expansion: PE-transpose each
         one-hot chunk, pd_c = ohT_c^T @ p_tile, kept in SBUF.
  AllGather(p); expand p likewise.
  MLP in slot order: gather p rows (same indices) -> ps; l = ps - pd + TeV
         (TeV = host-folded edge-attr term, like the baseline's TeS fold);
         2-class log_softmax via softplus: out = (-sp(-d), -sp(d)).
  Host drops self/pad slots and permutes the slot-ordered output back to
  original edge order (index bookkeeping only).

Algebraic collapses (validated against the reference numerically):
  h1 = relu(agg(norm*x) @ W1 + b1),  q = h1 @ (W2 @ fcW1[:64])  (b2 cancels)
  logits1 = p[s]-p[d] + TeV[e]; out = log_softmax(relu(logits1)@fcW2 + fcb2)
"""
import numpy as np
import ml_dtypes

import concourse.bacc as bacc
import concourse.bass as bass
import concourse.mybir as mybir
import concourse.tile as tile
from concourse.bass_utils import run_bass_kernel_spmd
from contextlib import ExitStack

dt = mybir.dt
bf16 = ml_dtypes.bfloat16
NCORES = 8
AF = mybir.ActivationFunctionType
ALU = mybir.AluOpType

N_NODES = 100000
XP = 100352          # padded nodes = 8 * 12544 = 4 * 25088
NSH = XP // NCORES   # 12544 nodes per core
QW = XP // 4         # 25088-row quarter windows (int16-safe)
TPC = NSH // 128     # 98 tiles per core
F = 64
MAXC = 32            # SBUF capacity in 128-slot chunks per tile


def _host_prep(x, edge_index, edge_attr, fcW1, fcb1, emb0, emb1):
    N = x.shape[0]
    src = np.asarray(edge_index[0], dtype=np.int64)
    dst = np.asarray(edge_index[1], dtype=np.int64)
    ea = np.asarray(edge_attr, dtype=np.int64)
    E = src.shape[0]

    deg = np.bincount(dst, minlength=XP).astype(np.float32) + 1.0
    dinv = 1.0 / np.sqrt(deg)

    fcW1 = np.asarray(fcW1, dtype=np.float32)
    Te0 = (np.asarray(emb0, dtype=np.float32) @ fcW1[66:98])
    Te1 = (np.asarray(emb1, dtype=np.float32) @ fcW1[98:130])
    TeV = (Te0[ea[2]] + Te1[ea[3]]
           + ea[0][:, None].astype(np.float32) * fcW1[64][None, :]
           + ea[1][:, None].astype(np.float32) * fcW1[65][None, :]
           + np.asarray(fcb1, dtype=np.float32)[None, :]).astype(np.float32)

    order = np.argsort(dst, kind="stable")
    s_sorted, d_sorted = src[order], dst[order]
    e_sorted = order
    ptr = np.searchsorted(d_sorted, np.arange(XP + 1))

    xpad = np.zeros((XP, F), dtype=np.float32)
    xpad[:N] = np.asarray(x, dtype=np.float32)

    # pass 1: per (core, tile, quarter) slot lists and counts
    lists = [[[None] * 4 for _ in range(TPC)] for _ in range(NCORES)]
    CQ = np.zeros((TPC, 4), dtype=np.int64)   # common chunk capacity
    for k in range(NCORES):
        lo = k * NSH
        for t in range(TPC):
            n0 = lo + t * 128
            n1 = n0 + 128
            a, b = ptr[n0], ptr[n1]
            ss = np.concatenate([s_sorted[a:b], np.arange(n0, n1)])
            dd = np.concatenate([d_sorted[a:b], np.arange(n0, n1)])
            ee = np.concatenate([e_sorted[a:b], np.full(128, -1, np.int64)])
            qt = ss // QW
            for q in range(4):
                m = qt == q
                lists[k][t][q] = (ss[m], dd[m], ee[m])
                CQ[t, q] = max(CQ[t, q], (int(m.sum()) + 127) // 128)
    assert CQ.sum(axis=1).max() <= 30

    # common structure
    instr_meta = []   # (t, q, Cq, off16)
    off16 = 0
    tile_C = CQ.sum(axis=1).astype(np.int64)
    for t in range(TPC):
        for q in range(4):
            if CQ[t, q] == 0:
                continue
            instr_meta.append((t, q, int(CQ[t, q]), off16))
            off16 += int(CQ[t, q]) * 8   # (Cq*128)/16 int16 cols
    NIDX16 = off16
    CSUM = int(tile_C.sum())
    coff = np.concatenate([[0], np.cumsum(tile_C)]).astype(np.int64)

    # pass 2: per-core arrays at common capacities
    percore = []
    slotmaps = []
    for k in range(NCORES):
        idx_all = np.zeros((128, NIDX16), dtype=np.int16)
        dstloc = np.full((128, CSUM), 300.0, dtype=np.float32)
        normv = np.zeros((128, CSUM), dtype=np.float32)
        tev = np.zeros((128, CSUM, 2), dtype=np.float32)
        smap = np.full((128, CSUM), -1, dtype=np.int64)
        for (t, q, Cq, o16) in instr_meta:
            ss, dd, ee = lists[k][t][q]
            cnt = len(ss)
            cap = Cq * 128
            n0 = k * NSH + t * 128
            idx = np.zeros(cap, dtype=np.int16)
            idx[:cnt] = (ss - q * QW).astype(np.int16)
            w = idx.reshape(cap // 16, 16).T
            for g in range(8):
                idx_all[g * 16:(g + 1) * 16, o16:o16 + cap // 16] = w
            # column base for this quarter inside the tile
            cq0 = int(coff[t]) + int(CQ[t, :q].sum())
            kk = np.arange(cnt)
            pp, cc = kk % 128, cq0 + kk // 128
            dstloc[pp, cc] = (dd - n0).astype(np.float32)
            normv[pp, cc] = dinv[ss] * dinv[dd]
            real = ee >= 0
            tev[pp[real], cc[real]] = TeV[ee[real]]
            smap[pp, cc] = ee
        percore.append(dict(
            x=xpad, idx16=idx_all,
            dstloc=np.ascontiguousarray(dstloc.astype(bf16)),
            normv=np.ascontiguousarray(normv),
            tev=np.ascontiguousarray(tev.reshape(128, CSUM * 2))))
        slotmaps.append(smap)

    meta = dict(instr_meta=instr_meta, tile_C=tile_C, coff=coff,
                CSUM=CSUM, NIDX16=NIDX16, E=E)
    return percore, slotmaps, meta


def _build(meta, fcW2, fcb2):
    instr_meta = meta["instr_meta"]
    tile_C = meta["tile_C"]
    coff = meta["coff"]
    CSUM = meta["CSUM"]
    NIDX16 = meta["NIDX16"]

    wd0 = float(fcW2[0, 0] - fcW2[0, 1])
    wd1 = float(fcW2[1, 0] - fcW2[1, 1])
    bd = float(fcb2[0] - fcb2[1])

    nc = bacc.Bacc("TRN2", target_bir_lowering=False, debug=False,
                   num_devices=NCORES, num_swdge_queues=4)

    t_x = nc.dram_tensor("x", [XP, F], dt.float32, kind="ExternalInput")
    t_idx = nc.dram_tensor("idx16", [128, NIDX16], dt.int16, kind="ExternalInput")
    t_dl = nc.dram_tensor("dstloc", [128, CSUM], dt.bfloat16, kind="ExternalInput")
    t_nv = nc.dram_tensor("normv", [128, CSUM], dt.float32, kind="ExternalInput")
    t_tev = nc.dram_tensor("tev", [128, CSUM * 2], dt.float32, kind="ExternalInput")
    t_W1b = nc.dram_tensor("W1b", [F, F], dt.bfloat16, kind="ExternalInput")
    t_W2nb = nc.dram_tensor("W2nb", [F, 2], dt.bfloat16, kind="ExternalInput")
    t_b1c = nc.dram_tensor("b1c", [F, 1], dt.float32, kind="ExternalInput")
    t_iota = nc.dram_tensor("iota128", [128, 128], dt.bfloat16, kind="ExternalInput")
    t_ident = nc.dram_tensor("ident128", [128, 128], dt.bfloat16, kind="ExternalInput")
    t_wd = nc.dram_tensor("wd", [128, 6], dt.float32, kind="ExternalInput")
    t_idf = nc.dram_tensor("idf", [2, 2], dt.float32, kind="ExternalInput")

    t_out = nc.dram_tensor("out", [128, CSUM * 2], dt.float32, kind="ExternalOutput")


    d_qloc = nc.dram_tensor("qloc_i", [NSH, 2], dt.float32)
    d_qfull = nc.dram_tensor("qfull_i", [XP, 2], dt.float32, addr_space="Shared")
    d_ploc = nc.dram_tensor("ploc_i", [NSH, 2], dt.float32)
    d_pfull = nc.dram_tensor("pfull_i", [XP, 2], dt.float32, addr_space="Shared")
    d_qexp = [nc.dram_tensor(f"qexp{i}", [QW, F], dt.float32) for i in range(4)]
    d_pexp = [nc.dram_tensor(f"pexp{i}", [QW, F], dt.float32) for i in range(4)]

    tile_instrs = [[] for _ in range(TPC)]
    for (t, q, Cq, o16) in instr_meta:
        tile_instrs[t].append((q, Cq, o16))

    with tile.TileContext(nc) as tc, ExitStack() as ctx:
        cst = ctx.enter_context(tc.tile_pool(name="cst", bufs=1))
        wk = ctx.enter_context(tc.tile_pool(name="wk", bufs=2))
        gp = ctx.enter_context(tc.tile_pool(name="gp", bufs=2))
        ohp = ctx.enter_context(tc.tile_pool(name="ohp", bufs=2))
        psA = ctx.enter_context(tc.tile_pool(name="psA", bufs=3, space="PSUM"))
        psB = ctx.enter_context(tc.tile_pool(name="psB", bufs=1, space="PSUM"))
        psC = ctx.enter_context(tc.tile_pool(name="psC", bufs=2, space="PSUM"))
        psD = ctx.enter_context(tc.tile_pool(name="psD", bufs=2, space="PSUM"))

        iota_t = cst.tile([128, 128], dt.bfloat16, tag="iota")
        nc.sync.dma_start(iota_t[:], t_iota[:, :])
        ident_t = cst.tile([128, 128], dt.bfloat16, tag="ident")
        nc.sync.dma_start(ident_t[:], t_ident[:, :])
        idf_t = cst.tile([2, 2], dt.float32, tag="idf")
        nc.sync.dma_start(idf_t[:], t_idf[:, :])
        W1b_t = cst.tile([F, F], dt.bfloat16, tag="W1b")
        nc.sync.dma_start(W1b_t[:], t_W1b[:, :])
        W2nb_t = cst.tile([F, 2], dt.bfloat16, tag="W2nb")
        nc.sync.dma_start(W2nb_t[:], t_W2nb[:, :])
        b1c_t = cst.tile([F, 1], dt.float32, tag="b1c")
        nc.sync.dma_start(b1c_t[:], t_b1c[:, :])
        wd_t = cst.tile([128, 6], dt.float32, tag="wd")
        nc.sync.dma_start(wd_t[:], t_wd[:, :])

        idx_t = cst.tile([128, NIDX16], dt.int16, tag="idx")
        nc.sync.dma_start(idx_t[:], t_idx[:, :])
        dl_t = cst.tile([128, CSUM], dt.bfloat16, tag="dl")
        nc.sync.dma_start(dl_t[:], t_dl[:, :])
        nv_t = cst.tile([128, CSUM], dt.float32, tag="nv")
        nc.sync.dma_start(nv_t[:], t_nv[:, :])
        tev_t = cst.tile([128, CSUM * 2], dt.float32, tag="tev")
        nc.sync.dma_start(tev_t[:], t_tev[:, :])
        pdv_t = cst.tile([128, CSUM * 2], dt.float32, tag="pdv")

        def gathers(t, table_list):
            g = gp.tile([128, MAXC * F], dt.float32, tag="g")
            co = 0
            for gi, (q, Cq, o16) in enumerate(tile_instrs[t]):
                n = Cq * 128
                tab = table_list[q]
                nc.gpsimd.dma_gather(
                    out_ap=g[:, co * F:(co + Cq) * F].rearrange(
                        "p (c f) -> p c f", f=F),
                    in_ap=tab if isinstance(tab, bass.AP) else tab[:, :],
                    idxs_ap=idx_t[:, o16:o16 + n // 16],
                    num_idxs=n, num_idxs_reg=n, elem_size=F,
                    queue_num=(t * 4 + gi) % 4)
                co += Cq
            return g

        def build_oh(t, C_t, c0):
            oh = ohp.tile([128, MAXC * 128], dt.bfloat16, tag="oh")
            nc.vector.tensor_tensor(
                out=oh[:, :C_t * 128].rearrange("p (c j) -> p c j", j=128),
                in0=dl_t[:, c0:c0 + C_t].broadcast_to((128, C_t, 128)),
                in1=iota_t[:].rearrange("p (o j) -> p o j", o=1).broadcast_to(
                    (128, C_t, 128)),
                op=ALU.is_equal)
            return oh

        # ---------------- conv1 ----------------
        for t in range(TPC):
            C_t = int(tile_C[t])
            c0 = int(coff[t])
            g = gathers(t, [t_x[i * QW:(i + 1) * QW, :] for i in range(4)])
            gbf = wk.tile([128, MAXC * F], dt.bfloat16, tag="gbf")
            nc.vector.tensor_tensor(
                out=gbf[:, :C_t * F].rearrange("p (c f) -> p c f", f=F),
                in0=g[:, :C_t * F].rearrange("p (c f) -> p c f", f=F),
                in1=nv_t[:, c0:c0 + C_t].rearrange(
                    "p (c o) -> p c o", o=1).broadcast_to((128, C_t, F)),
                op=ALU.mult)
            oh = build_oh(t, C_t, c0)
            aggT = psA.tile([F, 128], dt.float32, tag="p64")
            for c in range(C_t):
                nc.tensor.matmul(
                    out=aggT[:], lhsT=gbf[:, c * F:(c + 1) * F],
                    rhs=oh[:, c * 128:(c + 1) * 128],
                    start=(c == 0), stop=(c == C_t - 1))
            aggS = wk.tile([F, 128], dt.bfloat16, tag="aggS")
            nc.vector.tensor_copy(aggS[:], aggT[:])
            h1p = psA.tile([F, 128], dt.float32, tag="p64")
            nc.tensor.matmul(out=h1p[:], lhsT=W1b_t[:], rhs=aggS[:],
                             start=True, stop=True)
            h1r = wk.tile([F, 128], dt.bfloat16, tag="h1r")
            nc.scalar.activation(h1r[:], h1p[:], AF.Relu, bias=b1c_t[:, 0:1])
            qTp = psB.tile([2, 128], dt.float32, tag="p2")
            nc.tensor.matmul(out=qTp[:], lhsT=W2nb_t[:], rhs=h1r[:],
                             start=True, stop=True)
            qTs = wk.tile([2, 128], dt.float32, tag="qTs")
            nc.vector.tensor_copy(qTs[:], qTp[:])
            qmix = psC.tile([128, 64], dt.float32, tag="pMix")
            nc.tensor.transpose(out=qmix[:, 62:64], in_=qTs[:],
                                identity=idf_t[0:2, 0:2])
            qsb = wk.tile([128, 2], dt.float32, tag="qsb")
            nc.vector.tensor_copy(qsb[:], qmix[:, 62:64])
            nc.sync.dma_start(d_qloc[t * 128:(t + 1) * 128, 0:2], qsb[:])
        tc.strict_bb_all_engine_barrier()

        nc.gpsimd.collective_compute(
            "AllGather", ALU.bypass, replica_groups=[list(range(NCORES))],
            ins=[d_qloc[:, :].opt()], outs=[d_qfull[:, :].opt()])
        tc.strict_bb_all_engine_barrier()
        _eng = [nc.sync, nc.scalar, nc.sync, nc.scalar]
        for i in range(4):
            _eng[i].dma_start(d_qexp[i][:, 0:2], d_qfull[i * QW:(i + 1) * QW, :])
        tc.strict_bb_all_engine_barrier()

        # ---------------- conv2 + fused pd ----------------
        for t in range(TPC):
            C_t = int(tile_C[t])
            c0 = int(coff[t])
            g = gathers(t, d_qexp)
            g2n = wk.tile([128, MAXC * 2], dt.bfloat16, tag="g2n")
            nc.vector.tensor_tensor(
                out=g2n[:, :C_t * 2].rearrange("p (c w) -> p c w", w=2),
                in0=g[:, :C_t * F].rearrange("p (c f) -> p c f", f=F)[:, :, 0:2],
                in1=nv_t[:, c0:c0 + C_t].rearrange(
                    "p (c o) -> p c o", o=1).broadcast_to((128, C_t, 2)),
                op=ALU.mult)
            oh = build_oh(t, C_t, c0)
            aggT2 = psB.tile([2, 128], dt.float32, tag="p2")
            for c in range(C_t):
                nc.tensor.matmul(
                    out=aggT2[:], lhsT=g2n[:, c * 2:(c + 1) * 2],
                    rhs=oh[:, c * 128:(c + 1) * 128],
                    start=(c == 0), stop=(c == C_t - 1))
            a2s = wk.tile([2, 128], dt.float32, tag="a2s")
            nc.vector.tensor_copy(a2s[:], aggT2[:])
            pdp = psC.tile([128, 64], dt.float32, tag="pMix")
            nc.tensor.transpose(out=pdp[:, 62:64], in_=a2s[:],
                                identity=idf_t[0:2, 0:2])
            p_sb = wk.tile([128, 2], dt.float32, tag="psb")
            nc.vector.tensor_copy(p_sb[:], pdp[:, 62:64])
            nc.sync.dma_start(d_ploc[t * 128:(t + 1) * 128, 0:2], p_sb[:])
            p_bf = wk.tile([128, 2], dt.bfloat16, tag="pbf")
            nc.vector.tensor_copy(p_bf[:], pdp[:, 62:64])
            for c in range(C_t):
                ohTp = psD.tile([128, 128], dt.bfloat16, tag="ohTp")
                nc.tensor.transpose(out=ohTp[:], in_=oh[:, c * 128:(c + 1) * 128],
                                    identity=ident_t[:])
                ohTs = ohp.tile([128, 128], dt.bfloat16, tag="ohTs")
                # one-hot values are 0/1, so Relu == copy; runs on the idle ACT engine
                nc.scalar.activation(ohTs[:], ohTp[:], AF.Relu)
                nc.tensor.matmul(out=pdp[:, c * 2:(c + 1) * 2], lhsT=ohTs[:],
                                 rhs=p_bf[:], start=True, stop=True)
            nc.vector.tensor_copy(pdv_t[:, c0 * 2:(c0 + C_t) * 2],
                                  pdp[:, :C_t * 2])
        tc.strict_bb_all_engine_barrier()

        nc.gpsimd.collective_compute(
            "AllGather", ALU.bypass, replica_groups=[list(range(NCORES))],
            ins=[d_ploc[:, :].opt()], outs=[d_pfull[:, :].opt()])
        tc.strict_bb_all_engine_barrier()
        for i in range(4):
            _eng[i].dma_start(d_pexp[i][:, 0:2], d_pfull[i * QW:(i + 1) * QW, :])
        tc.strict_bb_all_engine_barrier()

        # ---------------- MLP (slot order) ----------------
        for t in range(TPC):
            C_t = int(tile_C[t])
            c0 = int(coff[t])
            g = gathers(t, d_pexp)
            ps3 = g[:, :C_t * F].rearrange("p (c f) -> p c f", f=F)[:, :, 0:2]
            l_ = wk.tile([128, MAXC * 2], dt.float32, tag="l")
            l3 = l_[:, :C_t * 2].rearrange("p (c w) -> p c w", w=2)
            nc.vector.tensor_tensor(
                out=l3, in0=ps3,
                in1=pdv_t[:, c0 * 2:(c0 + C_t) * 2].rearrange(
                    "p (c w) -> p c w", w=2),
                op=ALU.subtract)
            nc.vector.tensor_tensor(
                out=l_[:, :C_t * 2], in0=l_[:, :C_t * 2],
                in1=tev_t[:, c0 * 2:(c0 + C_t) * 2], op=ALU.add)
            r_ = wk.tile([128, MAXC * 2], dt.float32, tag="r")
            nc.scalar.activation(r_[:, :C_t * 2], l_[:, :C_t * 2], AF.Relu)
            rw = wk.tile([128, MAXC * 2], dt.float32, tag="rw")
            nc.vector.tensor_tensor(
                out=rw[:, :C_t * 2].rearrange("p (c w) -> p c w", w=2),
                in0=r_[:, :C_t * 2].rearrange("p (c w) -> p c w", w=2),
                in1=wd_t[:, 0:2].rearrange("p (o w) -> p o w", o=1).broadcast_to(
                    (128, C_t, 2)),
                op=ALU.mult)
            rw3 = rw[:, :C_t * 2].rearrange("p (c w) -> p c w", w=2)
            dd = wk.tile([128, MAXC], dt.float32, tag="dd")
            nc.vector.tensor_tensor(
                out=dd[:, :C_t].rearrange("p (c o) -> p c o", o=1),
                in0=rw3[:, :, 0:1], in1=rw3[:, :, 1:2], op=ALU.add)
            # delta = dd + bd; softplus(x) = relu(x) + ln(1 + exp(-|x|))
            dlt = wk.tile([128, MAXC], dt.float32, tag="dlt")
            nc.vector.tensor_scalar(out=dlt[:, :C_t], in0=dd[:, :C_t],
                                    scalar1=bd, scalar2=None, op0=ALU.add)
            ndl = wk.tile([128, MAXC], dt.float32, tag="ndl")
            nc.vector.tensor_scalar(out=ndl[:, :C_t], in0=dd[:, :C_t],
                                    scalar1=-1.0, scalar2=-bd,
                                    op0=ALU.mult, op1=ALU.add)
            # planar relu pair: rr = [relu(ndl) | relu(dlt)]
            rr = wk.tile([128, 2 * MAXC], dt.float32, tag="rr")
            nc.scalar.activation(rr[:, 0:C_t], ndl[:, :C_t], AF.Relu)
            nc.scalar.activation(rr[:, MAXC:MAXC + C_t], dlt[:, :C_t], AF.Relu)
            ab = wk.tile([128, MAXC], dt.float32, tag="ab")
            nc.vector.tensor_tensor(out=ab[:, :C_t], in0=dlt[:, :C_t],
                                    in1=ndl[:, :C_t], op=ALU.max)
            en = wk.tile([128, MAXC], dt.float32, tag="en")
            nc.scalar.activation(en[:, :C_t], ab[:, :C_t], AF.Exp,
                                 scale=wd_t[:, 5:6])
            lnp = wk.tile([128, MAXC], dt.float32, tag="lnp")
            nc.scalar.activation(lnp[:, :C_t], en[:, :C_t], AF.Ln,
                                 bias=1.0)
            # out planar: s = rr + lnp (broadcast over the 2 halves), negate
            so = wk.tile([128, 2 * MAXC], dt.float32, tag="so")
            nc.vector.tensor_tensor(
                out=so[:].rearrange("p (two c) -> p two c", two=2)[:, :, :C_t],
                in0=rr[:].rearrange("p (two c) -> p two c", two=2)[:, :, :C_t],
                in1=lnp[:, :C_t].rearrange("p (o c) -> p o c", o=1).broadcast_to(
                    (128, 2, C_t)),
                op=ALU.add)
            ot = wk.tile([128, 2 * MAXC], dt.float32, tag="ot")
            nc.vector.tensor_scalar(
                out=ot[:].rearrange("p (two c) -> p two c", two=2)[:, :, :C_t],
                in0=so[:].rearrange("p (two c) -> p two c", two=2)[:, :, :C_t],
                scalar1=-1.0, scalar2=None, op0=ALU.mult)
            nc.sync.dma_start(
                t_out[:, :].rearrange("p (two c) -> p two c", two=2)[:, :, c0:c0 + C_t],
                ot[:].rearrange("p (two c) -> p two c", two=2)[:, :, :C_t])

    nc.compile()
    return nc


def kernel(x, edge_index, edge_attr, W1, b1, W2, b2, emb0, emb1,
           fcW1, fcb1, fcW2, fcb2, _prep_only=False):
    x = np.asarray(x, dtype=np.float32)
    W1 = np.asarray(W1, dtype=np.float32)
    b1 = np.asarray(b1, dtype=np.float32)
    W2 = np.asarray(W2, dtype=np.float32)
    fcW1 = np.asarray(fcW1, dtype=np.float32)
    fcb1 = np.asarray(fcb1, dtype=np.float32)
    fcW2 = np.asarray(fcW2, dtype=np.float32)
    fcb2 = np.asarray(fcb2, dtype=np.float32)
    emb0 = np.asarray(emb0, dtype=np.float32)
    emb1 = np.asarray(emb1, dtype=np.float32)

    percore, slotmaps, meta = _host_prep(
        x, np.asarray(edge_index), np.asarray(edge_attr),
        fcW1, fcb1, emb0, emb1)

    W2n = (W2 @ fcW1[:64]).astype(np.float32)
    iota128 = np.broadcast_to(np.arange(128, dtype=np.float32),
                              (128, 128)).astype(bf16).copy()
    ident128 = np.eye(128, dtype=np.float32).astype(bf16)
    wd = np.zeros((128, 6), dtype=np.float32)
    wd[:, 0] = fcW2[0, 0] - fcW2[0, 1]
    wd[:, 1] = fcW2[1, 0] - fcW2[1, 1]
    wd[:, 2] = fcb2[0] - fcb2[1]
    wd[:, 3] = -(fcb2[0] - fcb2[1])
    wd[:, 4] = 1.0
    wd[:, 5] = -1.0
    for m in percore:
        m["W1b"] = W1.astype(bf16)
        m["W2nb"] = W2n.astype(bf16)
        m["b1c"] = b1.reshape(F, 1).astype(np.float32)
        m["iota128"] = iota128
        m["ident128"] = ident128
        m["wd"] = wd
        m["idf"] = np.eye(2, dtype=np.float32)

    nc = _build(meta, fcW2, fcb2)
    if _prep_only:
        return nc, percore, slotmaps, meta

    res = run_bass_kernel_spmd(nc, percore, core_ids=list(range(NCORES)))
    outs = [np.asarray(res.results[k]["out"]) for k in range(NCORES)]
    return assemble(outs, slotmaps, meta)


def assemble(outs, slotmaps, meta):
    E = meta["E"]
    CSUM = meta["CSUM"]
    result = np.zeros((E, 2), dtype=np.float32)
    for k in range(NCORES):
        o = outs[k].reshape(128, 2, CSUM).transpose(0, 2, 1)
        sm = slotmaps[k]
        mask = sm >= 0
        result[sm[mask]] = o[mask]
    return result


# revision 22
# speedup vs baseline: 1.0221x; 1.0221x over previous
"""GCN-with-edge-MLP kernel for trn2, 8-core SPMD (self-contained).

Equal node shards (12544 nodes/core, row = node id everywhere). Per core:
  conv1: per 128-node dst tile, slots (edges+self, grouped by src quarter
         window so indices fit int16) gathered from x fp32 rows (256B) via
         4-queue dma_gather, scaled by host norm (dinv_s*dinv_d) on the Pool
         engine, aggregated with one-hot (is_equal) matmuls on PE ->
         h1 = relu(agg@W1 + b1), q = h1 @ (W2@fcW1[:64]).
  AllGather(q); expand q into 256B-strided quarter tables (strided DMA).
  conv2: same slot structure and the SAME int16 indices, gather q rows,
         2-wide aggregation -> p. Fused pd expansion: PE-transpose each
         one-hot chunk, pd_c = ohT_c^T @ p_tile, kept in SBUF.
  AllGather(p); expand p likewise.
  MLP in slot order: gather p rows (same indices) -> ps; l = ps - pd + TeV
         (TeV = host-folded edge-attr term, like the baseline's TeS fold);
         2-class log_softmax via softplus: out = (-sp(-d), -sp(d)).
  Host drops self/pad slots and permutes the slot-ordered output back to
  original edge order (index bookkeeping only).

Algebraic collapses (validated against the reference numerically):
  h1 = relu(agg(norm*x) @ W1 + b1),  q = h1 @ (W2 @ fcW1[:64])  (b2 cancels)
  logits1 = p[s]-p[d] + TeV[e]; out = log_softmax(relu(logits1)@fcW2 + fcb2)
"""
import numpy as np
import ml_dtypes

import concourse.bacc as bacc
import concourse.bass as bass
import concourse.mybir as mybir
import concourse.tile as tile
from concourse.bass_utils import run_bass_kernel_spmd
from contextlib import ExitStack

dt = mybir.dt
bf16 = ml_dtypes.bfloat16
NCORES = 8
AF = mybir.ActivationFunctionType
ALU = mybir.AluOpType

N_NODES = 100000
XP = 100352          # padded nodes = 8 * 12544 = 4 * 25088
NSH = XP // NCORES   # 12544 nodes per core
QW = XP // 4         # 25088-row quarter windows (int16-safe)
TPC = NSH // 128     # 98 tiles per core
F = 64
MAXC = 32            # SBUF capacity in 128-slot chunks per tile


def _host_prep(x, edge_index, edge_attr, fcW1, fcb1, emb0, emb1):
    N = x.shape[0]
    src = np.asarray(edge_index[0], dtype=np.int64)
    dst = np.asarray(edge_index[1], dtype=np.int64)
    ea = np.asarray(edge_attr, dtype=np.int64)
    E = src.shape[0]

    deg = np.bincount(dst, minlength=XP).astype(np.float32) + 1.0
    dinv = 1.0 / np.sqrt(deg)

    fcW1 = np.asarray(fcW1, dtype=np.float32)
    Te0 = (np.asarray(emb0, dtype=np.float32) @ fcW1[66:98])
    Te1 = (np.asarray(emb1, dtype=np.float32) @ fcW1[98:130])
    TeV = (Te0[ea[2]] + Te1[ea[3]]
           + ea[0][:, None].astype(np.float32) * fcW1[64][None, :]
           + ea[1][:, None].astype(np.float32) * fcW1[65][None, :]
           + np.asarray(fcb1, dtype=np.float32)[None, :]).astype(np.float32)

    order = np.argsort(dst, kind="stable")
    s_sorted, d_sorted = src[order], dst[order]
    e_sorted = order
    ptr = np.searchsorted(d_sorted, np.arange(XP + 1))

    xpad = np.zeros((XP, F), dtype=np.float32)
    xpad[:N] = np.asarray(x, dtype=np.float32)

    # pass 1: per (core, tile, quarter) slot lists and counts
    lists = [[[None] * 4 for _ in range(TPC)] for _ in range(NCORES)]
    CQ = np.zeros((TPC, 4), dtype=np.int64)   # common chunk capacity
    for k in range(NCORES):
        lo = k * NSH
        for t in range(TPC):
            n0 = lo + t * 128
            n1 = n0 + 128
            a, b = ptr[n0], ptr[n1]
            ss = np.concatenate([s_sorted[a:b], np.arange(n0, n1)])
            dd = np.concatenate([d_sorted[a:b], np.arange(n0, n1)])
            ee = np.concatenate([e_sorted[a:b], np.full(128, -1, np.int64)])
            qt = ss // QW
            for q in range(4):
                m = qt == q
                lists[k][t][q] = (ss[m], dd[m], ee[m])
                CQ[t, q] = max(CQ[t, q], (int(m.sum()) + 127) // 128)
    assert CQ.sum(axis=1).max() <= 30

    # common structure
    instr_meta = []   # (t, q, Cq, off16)
    off16 = 0
    tile_C = CQ.sum(axis=1).astype(np.int64)
    for t in range(TPC):
        for q in range(4):
            if CQ[t, q] == 0:
                continue
            instr_meta.append((t, q, int(CQ[t, q]), off16))
            off16 += int(CQ[t, q]) * 8   # (Cq*128)/16 int16 cols
    NIDX16 = off16
    CSUM = int(tile_C.sum())
    coff = np.concatenate([[0], np.cumsum(tile_C)]).astype(np.int64)

    # pass 2: per-core arrays at common capacities
    percore = []
    slotmaps = []
    for k in range(NCORES):
        idx_all = np.zeros((128, NIDX16), dtype=np.int16)
        dstloc = np.full((128, CSUM), 300.0, dtype=np.float32)
        normv = np.zeros((128, CSUM), dtype=np.float32)
        tev = np.zeros((128, CSUM, 2), dtype=np.float32)
        smap = np.full((128, CSUM), -1, dtype=np.int64)
        for (t, q, Cq, o16) in instr_meta:
            ss, dd, ee = lists[k][t][q]
            cnt = len(ss)
            cap = Cq * 128
            n0 = k * NSH + t * 128
            idx = np.zeros(cap, dtype=np.int16)
            idx[:cnt] = (ss - q * QW).astype(np.int16)
            w = idx.reshape(cap // 16, 16).T
            for g in range(8):
                idx_all[g * 16:(g + 1) * 16, o16:o16 + cap // 16] = w
            # column base for this quarter inside the tile
            cq0 = int(coff[t]) + int(CQ[t, :q].sum())
            kk = np.arange(cnt)
            pp, cc = kk % 128, cq0 + kk // 128
            dstloc[pp, cc] = (dd - n0).astype(np.float32)
            normv[pp, cc] = dinv[ss] * dinv[dd]
            real = ee >= 0
            tev[pp[real], cc[real]] = TeV[ee[real]]
            smap[pp, cc] = ee
        percore.append(dict(
            x=xpad, idx16=idx_all,
            dstloc=np.ascontiguousarray(dstloc.astype(bf16)),
            normv=np.ascontiguousarray(normv),
            tev=np.ascontiguousarray(tev.reshape(128, CSUM * 2))))
        slotmaps.append(smap)

    meta = dict(instr_meta=instr_meta, tile_C=tile_C, coff=coff,
                CSUM=CSUM, NIDX16=NIDX16, E=E)
    return percore, slotmaps, meta


def _build(meta, fcW2, fcb2):
    instr_meta = meta["instr_meta"]
    tile_C = meta["tile_C"]
    coff = meta["coff"]
    CSUM = meta["CSUM"]
    NIDX16 = meta["NIDX16"]

    wd0 = float(fcW2[0, 0] - fcW2[0, 1])
    wd1 = float(fcW2[1, 0] - fcW2[1, 1])
    bd = float(fcb2[0] - fcb2[1])

    nc = bacc.Bacc("TRN2", target_bir_lowering=False, debug=False,
                   num_devices=NCORES, num_swdge_queues=4)

    t_x = nc.dram_tensor("x", [XP, F], dt.float32, kind="ExternalInput")
    t_idx = nc.dram_tensor("idx16", [128, NIDX16], dt.int16, kind="ExternalInput")
    t_dl = nc.dram_tensor("dstloc", [128, CSUM], dt.bfloat16, kind="ExternalInput")
    t_nv = nc.dram_tensor("normv", [128, CSUM], dt.float32, kind="ExternalInput")
    t_tev = nc.dram_tensor("tev", [128, CSUM * 2], dt.float32, kind="ExternalInput")
    t_W1b = nc.dram_tensor("W1b", [F, F], dt.bfloat16, kind="ExternalInput")
    t_W2nb = nc.dram_tensor("W2nb", [F, 2], dt.bfloat16, kind="ExternalInput")
    t_b1c = nc.dram_tensor("b1c", [F, 1], dt.float32, kind="ExternalInput")
    t_iota = nc.dram_tensor("iota128", [128, 128], dt.bfloat16, kind="ExternalInput")
    t_ident = nc.dram_tensor("ident128", [128, 128], dt.bfloat16, kind="ExternalInput")
    t_wd = nc.dram_tensor("wd", [128, 6], dt.float32, kind="ExternalInput")
    t_idf = nc.dram_tensor("idf", [2, 2], dt.float32, kind="ExternalInput")

    t_out = nc.dram_tensor("out", [128, CSUM * 2], dt.float32, kind="ExternalOutput")


    d_qloc = nc.dram_tensor("qloc_i", [NSH, 2], dt.float32)
    d_qfull = nc.dram_tensor("qfull_i", [XP, 2], dt.float32, addr_space="Shared")
    d_ploc = nc.dram_tensor("ploc_i", [NSH, 2], dt.float32)
    d_pfull = nc.dram_tensor("pfull_i", [XP, 2], dt.float32, addr_space="Shared")
    d_qexp = [nc.dram_tensor(f"qexp{i}", [QW, F], dt.float32) for i in range(4)]
    d_pexp = [nc.dram_tensor(f"pexp{i}", [QW, F], dt.float32) for i in range(4)]

    tile_instrs = [[] for _ in range(TPC)]
    for (t, q, Cq, o16) in instr_meta:
        tile_instrs[t].append((q, Cq, o16))

    with tile.TileContext(nc) as tc, ExitStack() as ctx:
        cst = ctx.enter_context(tc.tile_pool(name="cst", bufs=1))
        wk = ctx.enter_context(tc.tile_pool(name="wk", bufs=2))
        gp = ctx.enter_context(tc.tile_pool(name="gp", bufs=2))
        ohp = ctx.enter_context(tc.tile_pool(name="ohp", bufs=2))
        psA = ctx.enter_context(tc.tile_pool(name="psA", bufs=3, space="PSUM"))
        psB = ctx.enter_context(tc.tile_pool(name="psB", bufs=1, space="PSUM"))
        psC = ctx.enter_context(tc.tile_pool(name="psC", bufs=2, space="PSUM"))
        psD = ctx.enter_context(tc.tile_pool(name="psD", bufs=2, space="PSUM"))

        iota_t = cst.tile([128, 128], dt.bfloat16, tag="iota")
        nc.sync.dma_start(iota_t[:], t_iota[:, :])
        ident_t = cst.tile([128, 128], dt.bfloat16, tag="ident")
        nc.sync.dma_start(ident_t[:], t_ident[:, :])
        idf_t = cst.tile([2, 2], dt.float32, tag="idf")
        nc.sync.dma_start(idf_t[:], t_idf[:, :])
        W1b_t = cst.tile([F, F], dt.bfloat16, tag="W1b")
        nc.sync.dma_start(W1b_t[:], t_W1b[:, :])
        W2nb_t = cst.tile([F, 2], dt.bfloat16, tag="W2nb")
        nc.sync.dma_start(W2nb_t[:], t_W2nb[:, :])
        b1c_t = cst.tile([F, 1], dt.float32, tag="b1c")
        nc.sync.dma_start(b1c_t[:], t_b1c[:, :])
        wd_t = cst.tile([128, 6], dt.float32, tag="wd")
        nc.sync.dma_start(wd_t[:], t_wd[:, :])

        idx_t = cst.tile([128, NIDX16], dt.int16, tag="idx")
        nc.sync.dma_start(idx_t[:], t_idx[:, :])
        dl_t = cst.tile([128, CSUM], dt.bfloat16, tag="dl")
        nc.sync.dma_start(dl_t[:], t_dl[:, :])
        nv_t = cst.tile([128, CSUM], dt.float32, tag="nv")
        nc.sync.dma_start(nv_t[:], t_nv[:, :])
        tev_t = cst.tile([128, CSUM * 2], dt.float32, tag="tev")
        nc.sync.dma_start(tev_t[:], t_tev[:, :])
        pdv_t = cst.tile([128, CSUM * 2], dt.float32, tag="pdv")

        def gathers(t, table_list):
            g = gp.tile([128, MAXC * F], dt.float32, tag="g")
            co = 0
            for gi, (q, Cq, o16) in enumerate(tile_instrs[t]):
                n = Cq * 128
                tab = table_list[q]
                nc.gpsimd.dma_gather(
                    out_ap=g[:, co * F:(co + Cq) * F].rearrange(
                        "p (c f) -> p c f", f=F),
                    in_ap=tab if isinstance(tab, bass.AP) else tab[:, :],
                    idxs_ap=idx_t[:, o16:o16 + n // 16],
                    num_idxs=n, num_idxs_reg=n, elem_size=F,
                    queue_num=(t * 4 + gi) % 4)
                co += Cq
            return g

        def build_oh(t, C_t, c0):
            oh = ohp.tile([128, MAXC * 128], dt.bfloat16, tag="oh")
            nc.vector.tensor_tensor(
                out=oh[:, :C_t * 128].rearrange("p (c j) -> p c j", j=128),
                in0=dl_t[:, c0:c0 + C_t].broadcast_to((128, C_t, 128)),
                in1=iota_t[:].rearrange("p (o j) -> p o j", o=1).broadcast_to(
                    (128, C_t, 128)),
                op=ALU.is_equal)
            return oh

        # ---------------- conv1 ----------------
        for t in range(TPC):
            C_t = int(tile_C[t])
            c0 = int(coff[t])
            g = gathers(t, [t_x[i * QW:(i + 1) * QW, :] for i in range(4)])
            gbf = wk.tile([128, MAXC * F], dt.bfloat16, tag="gbf")
            nc.vector.tensor_tensor(
                out=gbf[:, :C_t * F].rearrange("p (c f) -> p c f", f=F),
                in0=g[:, :C_t * F].rearrange("p (c f) -> p c f", f=F),
                in1=nv_t[:, c0:c0 + C_t].rearrange(
                    "p (c o) -> p c o", o=1).broadcast_to((128, C_t, F)),
                op=ALU.mult)
            oh = build_oh(t, C_t, c0)
            aggT = psA.tile([F, 128], dt.float32, tag="p64")
            for c in range(C_t):
                nc.tensor.matmul(
                    out=aggT[:], lhsT=gbf[:, c * F:(c + 1) * F],
                    rhs=oh[:, c * 128:(c + 1) * 128],
                    start=(c == 0), stop=(c == C_t - 1))
            aggS = wk.tile([F, 128], dt.bfloat16, tag="aggS")
            nc.vector.tensor_copy(aggS[:], aggT[:])
            h1p = psA.tile([F, 128], dt.float32, tag="p64")
            nc.tensor.matmul(out=h1p[:], lhsT=W1b_t[:], rhs=aggS[:],
                             start=True, stop=True)
            h1r = wk.tile([F, 128], dt.bfloat16, tag="h1r")
            nc.scalar.activation(h1r[:], h1p[:], AF.Relu, bias=b1c_t[:, 0:1])
            qTp = psB.tile([2, 128], dt.float32, tag="p2")
            nc.tensor.matmul(out=qTp[:], lhsT=W2nb_t[:], rhs=h1r[:],
                             start=True, stop=True)
            qTs = wk.tile([2, 128], dt.float32, tag="qTs")
            nc.vector.tensor_copy(qTs[:], qTp[:])
            qmix = psC.tile([128, 64], dt.float32, tag="pMix")
            nc.tensor.transpose(out=qmix[:, 62:64], in_=qTs[:],
                                identity=idf_t[0:2, 0:2])
            qsb = wk.tile([128, 2], dt.float32, tag="qsb")
            nc.vector.tensor_copy(qsb[:], qmix[:, 62:64])
            nc.sync.dma_start(d_qloc[t * 128:(t + 1) * 128, 0:2], qsb[:])
        tc.strict_bb_all_engine_barrier()

        nc.gpsimd.collective_compute(
            "AllGather", ALU.bypass, replica_groups=[list(range(NCORES))],
            ins=[d_qloc[:, :].opt()], outs=[d_qfull[:, :].opt()])
        tc.strict_bb_all_engine_barrier()
        _eng = [nc.sync, nc.scalar, nc.sync, nc.scalar]
        for i in range(4):
            _eng[i].dma_start(d_qexp[i][:, 0:2], d_qfull[i * QW:(i + 1) * QW, :])
        tc.strict_bb_all_engine_barrier()

        # ---------------- conv2 + fused pd ----------------
        for t in range(TPC):
            C_t = int(tile_C[t])
            c0 = int(coff[t])
            g = gathers(t, d_qexp)
            g2n = wk.tile([128, MAXC * 2], dt.bfloat16, tag="g2n")
            nc.vector.tensor_tensor(
                out=g2n[:, :C_t * 2].rearrange("p (c w) -> p c w", w=2),
                in0=g[:, :C_t * F].rearrange("p (c f) -> p c f", f=F)[:, :, 0:2],
                in1=nv_t[:, c0:c0 + C_t].rearrange(
                    "p (c o) -> p c o", o=1).broadcast_to((128, C_t, 2)),
                op=ALU.mult)
            oh = build_oh(t, C_t, c0)
            aggT2 = psB.tile([2, 128], dt.float32, tag="p2")
            for c in range(C_t):
                nc.tensor.matmul(
                    out=aggT2[:], lhsT=g2n[:, c * 2:(c + 1) * 2],
                    rhs=oh[:, c * 128:(c + 1) * 128],
                    start=(c == 0), stop=(c == C_t - 1))
            a2s = wk.tile([2, 128], dt.float32, tag="a2s")
            nc.vector.tensor_copy(a2s[:], aggT2[:])
            pdp = psC.tile([128, 64], dt.float32, tag="pMix")
            nc.tensor.transpose(out=pdp[:, 62:64], in_=a2s[:],
                                identity=idf_t[0:2, 0:2])
            p_sb = wk.tile([128, 2], dt.float32, tag="psb")
            nc.vector.tensor_copy(p_sb[:], pdp[:, 62:64])
            nc.sync.dma_start(d_ploc[t * 128:(t + 1) * 128, 0:2], p_sb[:])
            p_bf = wk.tile([128, 2], dt.bfloat16, tag="pbf")
            nc.vector.tensor_copy(p_bf[:], pdp[:, 62:64])
            for c in range(C_t):
                ohTp = psD.tile([128, 128], dt.bfloat16, tag="ohTp")
                nc.tensor.transpose(out=ohTp[:], in_=oh[:, c * 128:(c + 1) * 128],
                                    identity=ident_t[:])
                ohTs = ohp.tile([128, 128], dt.bfloat16, tag="ohTs")
                nc.vector.tensor_copy(ohTs[:], ohTp[:])
                nc.tensor.matmul(out=pdp[:, c * 2:(c + 1) * 2], lhsT=ohTs[:],
                                 rhs=p_bf[:], start=True, stop=True)
            nc.vector.tensor_copy(pdv_t[:, c0 * 2:(c0 + C_t) * 2],
                                  pdp[:, :C_t * 2])
        tc.strict_bb_all_engine_barrier()

        nc.gpsimd.collective_compute(
            "AllGather", ALU.bypass, replica_groups=[list(range(NCORES))],
            ins=[d_ploc[:, :].opt()], outs=[d_pfull[:, :].opt()])
        tc.strict_bb_all_engine_barrier()
        for i in range(4):
            _eng[i].dma_start(d_pexp[i][:, 0:2], d_pfull[i * QW:(i + 1) * QW, :])
        tc.strict_bb_all_engine_barrier()

        # ---------------- MLP (slot order) ----------------
        for t in range(TPC):
            C_t = int(tile_C[t])
            c0 = int(coff[t])
            g = gathers(t, d_pexp)
            ps3 = g[:, :C_t * F].rearrange("p (c f) -> p c f", f=F)[:, :, 0:2]
            l_ = wk.tile([128, MAXC * 2], dt.float32, tag="l")
            l3 = l_[:, :C_t * 2].rearrange("p (c w) -> p c w", w=2)
            nc.vector.tensor_tensor(
                out=l3, in0=ps3,
                in1=pdv_t[:, c0 * 2:(c0 + C_t) * 2].rearrange(
                    "p (c w) -> p c w", w=2),
                op=ALU.subtract)
            nc.vector.tensor_tensor(
                out=l_[:, :C_t * 2], in0=l_[:, :C_t * 2],
                in1=tev_t[:, c0 * 2:(c0 + C_t) * 2], op=ALU.add)
            r_ = wk.tile([128, MAXC * 2], dt.float32, tag="r")
            nc.scalar.activation(r_[:, :C_t * 2], l_[:, :C_t * 2], AF.Relu)
            rw = wk.tile([128, MAXC * 2], dt.float32, tag="rw")
            nc.vector.tensor_tensor(
                out=rw[:, :C_t * 2].rearrange("p (c w) -> p c w", w=2),
                in0=r_[:, :C_t * 2].rearrange("p (c w) -> p c w", w=2),
                in1=wd_t[:, 0:2].rearrange("p (o w) -> p o w", o=1).broadcast_to(
                    (128, C_t, 2)),
                op=ALU.mult)
            rw3 = rw[:, :C_t * 2].rearrange("p (c w) -> p c w", w=2)
            dd = wk.tile([128, MAXC], dt.float32, tag="dd")
            nc.vector.tensor_tensor(
                out=dd[:, :C_t].rearrange("p (c o) -> p c o", o=1),
                in0=rw3[:, :, 0:1], in1=rw3[:, :, 1:2], op=ALU.add)
            # delta = dd + bd; softplus(x) = relu(x) + ln(1 + exp(-|x|))
            dlt = wk.tile([128, MAXC], dt.float32, tag="dlt")
            nc.vector.tensor_scalar(out=dlt[:, :C_t], in0=dd[:, :C_t],
                                    scalar1=bd, scalar2=None, op0=ALU.add)
            ndl = wk.tile([128, MAXC], dt.float32, tag="ndl")
            nc.vector.tensor_scalar(out=ndl[:, :C_t], in0=dd[:, :C_t],
                                    scalar1=-1.0, scalar2=-bd,
                                    op0=ALU.mult, op1=ALU.add)
            # planar relu pair: rr = [relu(ndl) | relu(dlt)]
            rr = wk.tile([128, 2 * MAXC], dt.float32, tag="rr")
            nc.scalar.activation(rr[:, 0:C_t], ndl[:, :C_t], AF.Relu)
            nc.scalar.activation(rr[:, MAXC:MAXC + C_t], dlt[:, :C_t], AF.Relu)
            ab = wk.tile([128, MAXC], dt.float32, tag="ab")
            nc.vector.tensor_tensor(out=ab[:, :C_t], in0=dlt[:, :C_t],
                                    in1=ndl[:, :C_t], op=ALU.max)
            en = wk.tile([128, MAXC], dt.float32, tag="en")
            nc.scalar.activation(en[:, :C_t], ab[:, :C_t], AF.Exp,
                                 scale=wd_t[:, 5:6])
            lnp = wk.tile([128, MAXC], dt.float32, tag="lnp")
            nc.scalar.activation(lnp[:, :C_t], en[:, :C_t], AF.Ln,
                                 bias=1.0)
            # out planar: s = rr + lnp (broadcast over the 2 halves), negate
            so = wk.tile([128, 2 * MAXC], dt.float32, tag="so")
            nc.vector.tensor_tensor(
                out=so[:].rearrange("p (two c) -> p two c", two=2)[:, :, :C_t],
                in0=rr[:].rearrange("p (two c) -> p two c", two=2)[:, :, :C_t],
                in1=lnp[:, :C_t].rearrange("p (o c) -> p o c", o=1).broadcast_to(
                    (128, 2, C_t)),
                op=ALU.add)
            ot = wk.tile([128, 2 * MAXC], dt.float32, tag="ot")
            nc.vector.tensor_scalar(
                out=ot[:].rearrange("p (two c) -> p two c", two=2)[:, :, :C_t],
                in0=so[:].rearrange("p (two c) -> p two c", two=2)[:, :, :C_t],
                scalar1=-1.0, scalar2=None, op0=ALU.mult)
            nc.sync.dma_start(
                t_out[:, :].rearrange("p (two c) -> p two c", two=2)[:, :, c0:c0 + C_t],
                ot[:].rearrange("p (two c) -> p two c", two=2)[:, :, :C_t])

    nc.compile()
    return nc


def kernel(x, edge_index, edge_attr, W1, b1, W2, b2, emb0, emb1,
           fcW1, fcb1, fcW2, fcb2, _prep_only=False):
    x = np.asarray(x, dtype=np.float32)
    W1 = np.asarray(W1, dtype=np.float32)
    b1 = np.asarray(b1, dtype=np.float32)
    W2 = np.asarray(W2, dtype=np.float32)
    fcW1 = np.asarray(fcW1, dtype=np.float32)
    fcb1 = np.asarray(fcb1, dtype=np.float32)
    fcW2 = np.asarray(fcW2, dtype=np.float32)
    fcb2 = np.asarray(fcb2, dtype=np.float32)
    emb0 = np.asarray(emb0, dtype=np.float32)
    emb1 = np.asarray(emb1, dtype=np.float32)

    percore, slotmaps, meta = _host_prep(
        x, np.asarray(edge_index), np.asarray(edge_attr),
        fcW1, fcb1, emb0, emb1)

    W2n = (W2 @ fcW1[:64]).astype(np.float32)
    iota128 = np.broadcast_to(np.arange(128, dtype=np.float32),
                              (128, 128)).astype(bf16).copy()
    ident128 = np.eye(128, dtype=np.float32).astype(bf16)
    wd = np.zeros((128, 6), dtype=np.float32)
    wd[:, 0] = fcW2[0, 0] - fcW2[0, 1]
    wd[:, 1] = fcW2[1, 0] - fcW2[1, 1]
    wd[:, 2] = fcb2[0] - fcb2[1]
    wd[:, 3] = -(fcb2[0] - fcb2[1])
    wd[:, 4] = 1.0
    wd[:, 5] = -1.0
    for m in percore:
        m["W1b"] = W1.astype(bf16)
        m["W2nb"] = W2n.astype(bf16)
        m["b1c"] = b1.reshape(F, 1).astype(np.float32)
        m["iota128"] = iota128
        m["ident128"] = ident128
        m["wd"] = wd
        m["idf"] = np.eye(2, dtype=np.float32)

    nc = _build(meta, fcW2, fcb2)
    if _prep_only:
        return nc, percore, slotmaps, meta

    res = run_bass_kernel_spmd(nc, percore, core_ids=list(range(NCORES)))
    outs = [np.asarray(res.results[k]["out"]) for k in range(NCORES)]
    return assemble(outs, slotmaps, meta)


def assemble(outs, slotmaps, meta):
    E = meta["E"]
    CSUM = meta["CSUM"]
    result = np.zeros((E, 2), dtype=np.float32)
    for k in range(NCORES):
        o = outs[k].reshape(128, 2, CSUM).transpose(0, 2, 1)
        sm = slotmaps[k]
        mask = sm >= 0
        result[sm[mask]] = o[mask]
    return result
